# revision 14
# baseline (speedup 1.0000x reference)
"""De-emphasis IIR filter y[n] = c*y[n-1] + x[n] (c=0.95) on 8 NeuronCores.

Input: (64, 524288) fp32. Pure data parallel: 8 rows per core.

Per row: reshape to [128 partitions x 4096 cols]; partition p holds the
contiguous sample chunk [p*4096, (p+1)*4096). Each partition runs the
recurrence along its free dim with the native DVE tensor_tensor_scan
instruction. Because c^4096 underflows fp32 (~1e-91), the true carry into
partition p is exactly the last element of partition p-1's *local* scan --
no sequential chain across partitions. The carry vector is shifted down
one partition via a tiny matmul with a superdiagonal 0/1 matrix on the
(otherwise idle) TensorEngine, and the first W=384 columns are re-scanned
with initial=carry (c^385 ~ 3e-9, far below fp32 resolution).

Rows are processed two at a time ([128, 2, 4096] tiles): the core runs on
exactly 8 large DMAs (4 in + 4 out, 4 MiB each) so each of the 8 rotating
HWDGE completion semaphores is used at most once (no lane-reuse ordering
waits). Inputs are triple-buffered and the DMA FIFO order is pinned with
no-sync edges (x0,x1,x2,y0,x3,y1,y2,y3) so input prefetch stays ahead of
outputs. This walrus build allows only ONE semaphore wait per engine
datapath instruction; 1-element DVE "touch" copies absorb cross-engine
waits ahead of the scans, and a chain of single-wait sequencer nops
observes every proc's final tick so the auto-generated kernel-tail drain
needs no waits of its own.
"""

import os
import sys

import numpy as np

if "/opt/trn_rl_repo" not in sys.path:
    sys.path.insert(0, "/opt/trn_rl_repo")

import concourse.bass as bass
import concourse.mybir as mybir
from concourse import tile
from concourse.tile import add_dep_helper
from concourse.bass_utils import run_bass_kernel_spmd

N_CORES = 8
BATCH = 64
T = 524288
P = 128              # SBUF partitions
L = T // P           # 4096 columns per row
SUB = 2              # rows per pair-tile
PAIRS = BATCH // N_CORES // SUB  # 4 pair-tiles per core
W = 384              # carry-correction width (c^(W+1) ~ 3e-9)
COEFF = 0.95

LAST_EXEC_TIME_NS = None

_nc_cache = None

F32 = mybir.dt.float32
MULT = mybir.AluOpType.mult
ADD = mybir.AluOpType.add


def build_nc(pairs=PAIRS, cols=L, width=W, coeff=COEFF):
    nc = bass.Bass()
    x_d = nc.declare_dram_parameter("x", [pairs, SUB, P, cols], F32, isOutput=False)
    y_d = nc.declare_dram_parameter("y", [pairs, SUB, P, cols], F32, isOutput=True)

    dma_chain = []   # all DMAs in pinned FIFO order

    def chain_dma(inst):
        if dma_chain:
            add_dep_helper(inst.ins, dma_chain[-1].ins, sync=False,
                           reason="pin SP DMA FIFO order")
        dma_chain.append(inst)
        return inst

    with tile.TileContext(nc) as tc:
        with (
            tc.tile_pool(name="consts", bufs=1) as cpool,
            tc.tile_pool(name="xin", bufs=3) as xpool,
            tc.tile_pool(name="yout", bufs=2) as ypool,
            tc.tile_pool(name="carrysb", bufs=2) as spool,
            tc.tile_pool(name="carry", bufs=2, space="PSUM") as ppool,
        ):
            # Coefficient tile on DVE so scans depend on it same-engine.
            c_tile = cpool.tile([P, cols], F32)
            nc.vector.memset(c_tile[:], coeff)

            # Superdiagonal shift matrix S[k, k+1] = 1, built on GPSIMD
            # (iota-family ops live there), bounced through a DVE copy so
            # every matmul's deps collapse onto the DVE semaphore.
            ones = cpool.tile([P, P], F32)
            nc.gpsimd.memset(ones[:], 1.0)
            s_g = cpool.tile([P, P], F32)
            # select ones where (m - k - 1) == 0 else 0.0
            s_g_inst = nc.gpsimd.affine_select(
                s_g[:], ones[:], pattern=[[1, P]],
                compare_op=mybir.AluOpType.is_equal,
                fill=0.0, base=-1, channel_multiplier=-1,
            )
            s_tile = cpool.tile([P, P], F32)
            nc.vector.tensor_copy(s_tile[:], s_g[:])

            scratch = cpool.tile([P, 1], F32)

            # ---- prefetch inputs for pairs 0..2 (triple-buffered) ----
            x_tiles = [xpool.tile([P, SUB, cols], F32, name=f"xt{j}", tag="xt")
                       for j in range(pairs)]
            xin = [None] * pairs
            for i in range(3):
                xin[i] = chain_dma(nc.sync.dma_start(
                    x_tiles[i][:], x_d[i].rearrange("s p l -> p s l")))
                if i > 0:
                    # completion-chain the prefetches: x0 alone gets full
                    # SDMA bandwidth, so the first scan starts ~4us sooner
                    # (one wait each -- these DMAs had none).
                    add_dep_helper(xin[i].ins, xin[i - 1].ins,
                                   reason="serialize prefetch stream")

            yout = [None] * pairs
            last_dve = [None] * pairs

            for i in range(pairs):
                x_t = x_tiles[i]
                y_t = ypool.tile([P, SUB, cols], F32)

                # DVE datapath touches (engine-proc, so scans inherit their
                # observed ticks), one cross-engine wait each. tx absorbs
                # the x-in completion; ty absorbs the y-slot recycle WAR
                # (reads c_tile, whose tick is long observed, to avoid a
                # same-engine pipeline-hazard wait on scratch).
                nc.vector.tensor_copy(scratch[0:1, 0:1], x_t[0:1, 0, 0:1])
                if i >= 2:
                    nc.vector.tensor_copy(y_t[0:1, 0, 0:1], c_tile[0:1, 0:1])
                nc.vector.tensor_tensor_scan(
                    y_t[:, 0, :], c_tile[:], x_t[:, 0, :], 0.0, MULT, ADD)
                nc.vector.tensor_tensor_scan(
                    y_t[:, 1, :], c_tile[:], x_t[:, 1, :], 0.0, MULT, ADD)

                carry = ppool.tile([P, SUB], F32)
                mm_inst = nc.tensor.matmul(
                    carry[:], s_tile[:], y_t[:, :, cols - 1],
                    start=True, stop=True,
                )
                carry_sb = spool.tile([P, SUB], F32)
                nc.vector.tensor_copy(carry_sb[:], carry[:])

                nc.vector.tensor_tensor_scan(
                    y_t[:, 0, 0:width], c_tile[:, 0:width], x_t[:, 0, 0:width],
                    carry_sb[:, 0:1], MULT, ADD)
                s2b = nc.vector.tensor_tensor_scan(
                    y_t[:, 1, 0:width], c_tile[:, 0:width], x_t[:, 1, 0:width],
                    carry_sb[:, 1:2], MULT, ADD)
                last_dve[i] = s2b

                yout[i] = chain_dma(nc.sync.dma_start(
                    y_d[i].rearrange("s p l -> p s l"), y_t[:]))
                if i == 0 and pairs > 3:
                    # prefetch the last input right after the first output:
                    # its WAR (scan2b(0), DVE) is already observed via y0's
                    # wait, leaving only the x0-WAW lane wait -- one wait.
                    xin[3] = chain_dma(nc.sync.dma_start(
                        x_tiles[3][:], x_d[3].rearrange("s p l -> p s l")))

            # Tail absorbers: the auto-generated kernel-tail drain waits on
            # every proc with an unobserved final tick; observe each final
            # tick on single-wait SP nops so the drain needs none.
            tail_deps = [s_g_inst, mm_inst, last_dve[pairs - 1]]
            tail_deps += [d for d in xin if d is not None]
            tail_deps += [d for d in yout if d is not None]
            prev = None
            for k, dep in enumerate(tail_deps):
                tn = nc.sync.nop(hint=f"tail{k}", nofuse=True)
                add_dep_helper(tn.ins, dep.ins, reason="tail drain absorb")
                if prev is not None:
                    add_dep_helper(tn.ins, prev.ins, sync=False,
                                   reason="tail chain order")
                prev = tn
    return nc


def kernel(inputs: np.ndarray) -> np.ndarray:
    global LAST_EXEC_TIME_NS, _nc_cache
    x = np.ascontiguousarray(inputs, dtype=np.float32)
    assert x.shape == (BATCH, T), x.shape
    if _nc_cache is None:
        _nc_cache = build_nc()
    nc = _nc_cache
    rows_per_core = BATCH // N_CORES
    in_maps = [
        {"x": x[k * rows_per_core : (k + 1) * rows_per_core].reshape(PAIRS, SUB, P, L)}
        for k in range(N_CORES)
    ]
    res = run_bass_kernel_spmd(nc, in_maps, list(range(N_CORES)))
    LAST_EXEC_TIME_NS = res.exec_time_ns
    return np.concatenate(
        [res.results[k]["y"].reshape(rows_per_core, T) for k in range(N_CORES)],
        axis=0,
    )


# revision 15
# speedup vs baseline: 1.0807x; 1.0807x over previous
"""De-emphasis IIR filter y[n] = c*y[n-1] + x[n] (c=0.95) on 8 NeuronCores.

Input: (64, 524288) fp32. Pure data parallel: 8 rows per core.

Per row: reshape to [128 partitions x 4096 cols]; partition p holds the
contiguous sample chunk [p*4096, (p+1)*4096). Each partition runs the
recurrence along its free dim with the native DVE tensor_tensor_scan
instruction. Because c^4096 underflows fp32 (~1e-91), the true carry into
partition p is exactly the last element of partition p-1's *local* scan --
no sequential chain across partitions. The carry vector is shifted down
one partition via a tiny matmul with a superdiagonal 0/1 matrix on the
(otherwise idle) TensorEngine, and the first W=384 columns are re-scanned
with initial=carry (c^385 ~ 3e-9, far below fp32 resolution).

Rows are processed two at a time ([128, 2, 4096] tiles): the core runs on
exactly 8 large DMAs (4 in + 4 out, 4 MiB each) so each of the 8 rotating
HWDGE completion semaphores is used at most once (no lane-reuse ordering
waits). Inputs are triple-buffered and the DMA FIFO order is pinned with
no-sync edges (x0,x1,x2,y0,x3,y1,y2,y3) so input prefetch stays ahead of
outputs. This walrus build allows only ONE semaphore wait per engine
datapath instruction; 1-element DVE "touch" copies absorb cross-engine
waits ahead of the scans, and a chain of single-wait sequencer nops
observes every proc's final tick so the auto-generated kernel-tail drain
needs no waits of its own.
"""

import os
import sys

import numpy as np

if "/opt/trn_rl_repo" not in sys.path:
    sys.path.insert(0, "/opt/trn_rl_repo")

import concourse.bass as bass
import concourse.mybir as mybir
from concourse import tile
from concourse.tile import add_dep_helper
from concourse.bass_utils import run_bass_kernel_spmd

N_CORES = 8
BATCH = 64
T = 524288
P = 128              # SBUF partitions
L = T // P           # 4096 columns per row
SUB = 2              # rows per pair-tile
PAIRS = BATCH // N_CORES // SUB  # 4 pair-tiles per core
W = 384              # carry-correction width (c^(W+1) ~ 3e-9)
COEFF = 0.95

LAST_EXEC_TIME_NS = None

_nc_cache = None

F32 = mybir.dt.float32
MULT = mybir.AluOpType.mult
ADD = mybir.AluOpType.add


def build_nc(pairs=PAIRS, cols=L, width=W, coeff=COEFF):
    nc = bass.Bass()
    x_d = nc.declare_dram_parameter("x", [pairs, SUB, P, cols], F32, isOutput=False)
    y_d = nc.declare_dram_parameter("y", [pairs, SUB, P, cols], F32, isOutput=True)

    dma_chain = []   # all DMAs in pinned FIFO order

    def chain_dma(inst):
        if dma_chain:
            add_dep_helper(inst.ins, dma_chain[-1].ins, sync=False,
                           reason="pin SP DMA FIFO order")
        dma_chain.append(inst)
        return inst

    with tile.TileContext(nc) as tc:
        with (
            tc.tile_pool(name="consts", bufs=1) as cpool,
            tc.tile_pool(name="xin", bufs=3) as xpool,
            tc.tile_pool(name="yout", bufs=2) as ypool,
            tc.tile_pool(name="carrysb", bufs=2) as spool,
            tc.tile_pool(name="carry", bufs=2, space="PSUM") as ppool,
        ):
            # Coefficient tile on DVE so scans depend on it same-engine.
            c_tile = cpool.tile([P, cols], F32)
            nc.vector.memset(c_tile[:], coeff)

            # Superdiagonal shift matrix S[k, k+1] = 1, built on GPSIMD
            # (iota-family ops live there), bounced through a DVE copy so
            # every matmul's deps collapse onto the DVE semaphore.
            ones = cpool.tile([P, P], F32)
            nc.gpsimd.memset(ones[:], 1.0)
            s_g = cpool.tile([P, P], F32)
            # select ones where (m - k - 1) == 0 else 0.0
            s_g_inst = nc.gpsimd.affine_select(
                s_g[:], ones[:], pattern=[[1, P]],
                compare_op=mybir.AluOpType.is_equal,
                fill=0.0, base=-1, channel_multiplier=-1,
            )
            s_tile = cpool.tile([P, P], F32)
            nc.vector.tensor_copy(s_tile[:], s_g[:])

            scratch = cpool.tile([P, 1], F32)

            # ---- prefetch inputs for pairs 0..2 (triple-buffered) ----
            x_tiles = [xpool.tile([P, SUB, cols], F32, name=f"xt{j}", tag="xt")
                       for j in range(pairs)]
            xin = [None] * pairs
            for i in range(3):
                xin[i] = chain_dma(nc.sync.dma_start(
                    x_tiles[i][:], x_d[i].rearrange("s p l -> p s l")))

            yout = [None] * pairs
            last_dve = [None] * pairs

            for i in range(pairs):
                x_t = x_tiles[i]
                y_t = ypool.tile([P, SUB, cols], F32)

                # DVE datapath touches (engine-proc, so scans inherit their
                # observed ticks), one cross-engine wait each. tx absorbs
                # the x-in completion; ty absorbs the y-slot recycle WAR
                # (reads c_tile, whose tick is long observed, to avoid a
                # same-engine pipeline-hazard wait on scratch).
                nc.vector.tensor_copy(scratch[0:1, 0:1], x_t[0:1, 0, 0:1])
                if i >= 2:
                    nc.vector.tensor_copy(y_t[0:1, 0, 0:1], c_tile[0:1, 0:1])
                nc.vector.tensor_tensor_scan(
                    y_t[:, 0, :], c_tile[:], x_t[:, 0, :], 0.0, MULT, ADD)
                nc.vector.tensor_tensor_scan(
                    y_t[:, 1, :], c_tile[:], x_t[:, 1, :], 0.0, MULT, ADD)

                carry = ppool.tile([P, SUB], F32)
                mm_inst = nc.tensor.matmul(
                    carry[:], s_tile[:], y_t[:, :, cols - 1],
                    start=True, stop=True,
                )
                carry_sb = spool.tile([P, SUB], F32)
                nc.vector.tensor_copy(carry_sb[:], carry[:])

                nc.vector.tensor_tensor_scan(
                    y_t[:, 0, 0:width], c_tile[:, 0:width], x_t[:, 0, 0:width],
                    carry_sb[:, 0:1], MULT, ADD)
                s2b = nc.vector.tensor_tensor_scan(
                    y_t[:, 1, 0:width], c_tile[:, 0:width], x_t[:, 1, 0:width],
                    carry_sb[:, 1:2], MULT, ADD)
                last_dve[i] = s2b

                yout[i] = chain_dma(nc.sync.dma_start(
                    y_d[i].rearrange("s p l -> p s l"), y_t[:]))
                if i == 0 and pairs > 3:
                    # prefetch the last input right after the first output:
                    # its WAR (scan2b(0), DVE) is already observed via y0's
                    # wait, leaving only the x0-WAW lane wait -- one wait.
                    xin[3] = chain_dma(nc.sync.dma_start(
                        x_tiles[3][:], x_d[3].rearrange("s p l -> p s l")))

            # Tail absorbers: the auto-generated kernel-tail drain waits on
            # every proc with an unobserved final tick; observe each final
            # tick on single-wait SP nops so the drain needs none.
            tail_deps = [s_g_inst, mm_inst, last_dve[pairs - 1]]
            tail_deps += [d for d in xin if d is not None]
            tail_deps += [d for d in yout if d is not None]
            prev = None
            for k, dep in enumerate(tail_deps):
                tn = nc.sync.nop(hint=f"tail{k}", nofuse=True)
                add_dep_helper(tn.ins, dep.ins, reason="tail drain absorb")
                if prev is not None:
                    add_dep_helper(tn.ins, prev.ins, sync=False,
                                   reason="tail chain order")
                prev = tn
    return nc


def kernel(inputs: np.ndarray) -> np.ndarray:
    global LAST_EXEC_TIME_NS, _nc_cache
    x = np.ascontiguousarray(inputs, dtype=np.float32)
    assert x.shape == (BATCH, T), x.shape
    if _nc_cache is None:
        _nc_cache = build_nc()
    nc = _nc_cache
    rows_per_core = BATCH // N_CORES
    in_maps = [
        {"x": x[k * rows_per_core : (k + 1) * rows_per_core].reshape(PAIRS, SUB, P, L)}
        for k in range(N_CORES)
    ]
    res = run_bass_kernel_spmd(nc, in_maps, list(range(N_CORES)))
    LAST_EXEC_TIME_NS = res.exec_time_ns
    return np.concatenate(
        [res.results[k]["y"].reshape(rows_per_core, T) for k in range(N_CORES)],
        axis=0,
    )
